# revision 1
# baseline (speedup 1.0000x reference)
"""Trainium2 Bass kernel for a 3-layer BiLSTM + ReLU + residual + LayerNorm.

Strategy (pure data parallel over 8 cores, 1024 batch rows per core):
  * "Transposed" layout on-chip: features on SBUF partitions, batch on the
    free dim.  Both directions fused on partitions (fwd = 0:64, bwd = 64:128)
    so every ScalarE/VectorE op runs with all 128 lanes busy.
  * Per timestep, per gate: one 128x128 block-diagonal recurrent matmul plus
    two 64-wide input-projection matmuls (col-tiled pairs) accumulate into a
    PSUM tile [g_fwd; g_bwd].  Sigmoid over a contiguous [i,f,o] PSUM span is
    a single ScalarE op; tanh(g), the c/h updates, and tanh(c) follow.
  * Layer outputs stream through DRAM as bf16 (the recurrence itself stays
    fp32); batch is processed as two interleaved chunks so the engines can
    overlap the sequential per-step dependency chain.
  * Final stage: PE transpose to natural layout + K=9 residual matmul into
    the same PSUM tile, LayerNorm stats via ScalarE accumulate outputs,
    normalize via per-partition tensor_scalar, DMA out natural-layout fp32.
"""

from contextlib import ExitStack

import numpy as np
import ml_dtypes

import concourse.bacc as bacc
import concourse.tile as tile
from concourse import mybir
from concourse.bass_utils import run_bass_kernel_spmd

F32 = mybir.dt.float32
BF16 = mybir.dt.bfloat16
AF = mybir.ActivationFunctionType
OP = mybir.AluOpType

NCORES = 8
BC = 1024               # batch rows per core
CHUNKS = 2
T = 64
H = 64
NL = 3
D2 = 2 * H              # 128
LN_EPS = 1e-5

# gate order in PyTorch weights: i, f, g, o  (rows g*H:(g+1)*H of w_ih/w_hh)
SIG_GATES = (0, 1, 3)   # i, f, o  -> sigmoid, held in one PSUM span
TANH_GATE = 2           # g        -> tanh


def _host_prep(x, w_ih, w_hh, b_ih, b_hh, w_res, b_res, ncores, bc):
    """Matmul-ready weight layouts (shared across cores) + per-core inputs."""
    x = np.asarray(x, np.float32)
    w_ih = np.asarray(w_ih, np.float32)
    w_hh = np.asarray(w_hh, np.float32)
    bias = np.asarray(b_ih, np.float32) + np.asarray(b_hh, np.float32)  # (NL,2,4H)
    w_res = np.asarray(w_res, np.float32)
    b_res = np.asarray(b_res, np.float32)
    t_len = x.shape[1]

    # Recurrent lhsT, K-major: rw[k, l, g, m] (block-diagonal over directions)
    rw = np.zeros((128, NL, 4, 128), np.float32)
    for l in range(NL):
        for g in range(4):
            gs = slice(g * H, (g + 1) * H)
            rw[0:64, l, g, 0:64] = w_hh[l, 0, gs, :].T
            rw[64:128, l, g, 64:128] = w_hh[l, 1, gs, :].T

    # Input-projection lhsT for layers 1,2 (bf16): pw[k, l-1, g, d, m]
    pw = np.zeros((128, NL - 1, 4, 2, 64), np.float32)
    for l in (1, 2):
        for g in range(4):
            gs = slice(g * H, (g + 1) * H)
            for d in range(2):
                pw[:, l - 1, g, d, :] = w_ih[l, d, gs, :].T
    pw = pw.astype(ml_dtypes.bfloat16)

    # Layer-0 projection lhsT with the bias folded into a ones-row (row 8)
    l0w = np.zeros((9, 4, 2, 64), np.float32)
    for g in range(4):
        gs = slice(g * H, (g + 1) * H)
        for d in range(2):
            l0w[0:8, g, d, :] = w_ih[0, d, gs, 0:8].T
            l0w[8, g, d, :] = bias[0, d, gs]

    # per-partition sigmoid-gate biases, layers 1,2 (fused dirs): br[p, idx]
    br = np.zeros((128, (NL - 1) * 3), np.float32)
    for l in (1, 2):
        for j, g in enumerate(SIG_GATES):
            gs = slice(g * H, (g + 1) * H)
            br[0:64, (l - 1) * 3 + j] = bias[l, 0, gs]
            br[64:128, (l - 1) * 3 + j] = bias[l, 1, gs]

    # g-gate bias per layer 1,2, per fused partition: gb[p, l-1]
    gb = np.zeros((128, NL - 1), np.float32)
    gs = slice(TANH_GATE * H, (TANH_GATE + 1) * H)
    for l in (1, 2):
        gb[0:64, l - 1] = bias[l, 0, gs]
        gb[64:128, l - 1] = bias[l, 1, gs]

    # residual rhs: wres[k, f] = w_res[f, k], row 8 = b_res
    wres = np.zeros((9, 128), np.float32)
    wres[0:8, :] = w_res.T
    wres[8, :] = b_res

    ident = np.eye(128, dtype=np.float32)

    # Per-core transposed-augmented input: xaug[k, t, b]
    xaug_cores = []
    for c in range(ncores):
        xc = x[c * bc:(c + 1) * bc]              # (bc, T, 8)
        xa = np.empty((9, t_len, bc), np.float32)
        xa[0:8] = xc.transpose(2, 1, 0)
        xa[8] = 1.0
        xaug_cores.append(xa)

    shared = dict(rw=rw, pw=pw, l0w=l0w, br=br, gb=gb, wres=wres, ident=ident)
    return shared, xaug_cores


def _emit(nc, tc, ctx, D, apply_gb, bc, t_len):
    bk = bc // CHUNKS
    fb = min(128, bk)         # final-stage block width (natural-layout rows)
    nb = bk // fb             # blocks per chunk per timestep
    strip = min(8, t_len)

    sbC = ctx.enter_context(tc.tile_pool(name="consts", bufs=1))
    sbA = ctx.enter_context(tc.tile_pool(name="workA", bufs=3))
    sbB = ctx.enter_context(tc.tile_pool(name="workB", bufs=2))
    sbS = ctx.enter_context(tc.tile_pool(name="state", bufs=1))
    sbZ = ctx.enter_context(tc.tile_pool(name="zhold", bufs=strip + 2))
    ps = ctx.enter_context(tc.tile_pool(name="ps", bufs=1, space="PSUM"))

    def const_tile(shape, dtype, key):
        t = sbC.tile(shape, dtype, name=f"c_{key}", tag=f"c_{key}")
        nc.sync.dma_start(out=t, in_=D[key])
        return t

    rw_sb = const_tile([128, NL, 4, 128], F32, "rw")
    pw_sb = const_tile([128, NL - 1, 4, 2, 64], BF16, "pw")
    l0w_sb = const_tile([9, 4, 2, 64], F32, "l0w")
    br_sb = const_tile([128, (NL - 1) * 3], F32, "br")
    gb_sb = const_tile([128, NL - 1], F32, "gb")
    wres_sb = const_tile([9, 128], F32, "wres")
    ident_sb = const_tile([128, 128], F32, "ident")
    gamma_sb = beta_sb = None
    if apply_gb:
        gamma_sb = const_tile([fb, 128], F32, "gammab")
        beta_sb = const_tile([fb, 128], F32, "betab")
    ones_sb = sbC.tile([1, bk], F32)
    nc.vector.memset(ones_sb, 1.0)
    eps_sb = sbC.tile([128, 1], F32)
    nc.vector.memset(eps_sb, LN_EPS)

    O = [D[f"o{i}"] for i in range(NL)]
    xaug = D["xaug"]
    out_d = D["out"]

    h_prev = [None] * CHUNKS
    c_st = [None] * CHUNKS

    def issue_inp(cc, l, k):
        # issued ahead of the consuming step so input reads enter the DMA
        # queue before the chain-tail output writes (no head-of-line block)
        c0 = cc * bk
        cols = slice(c0, c0 + bk)
        rt = t_len - 1 - k
        if l == 0:
            inp_f = sbA.tile([9, bk], F32, tag=f"inf{cc}", bufs=4, name="inp_f")
            nc.sync.dma_start(out=inp_f, in_=xaug[:, k, cols])
            inp_b = sbA.tile([9, bk], F32, tag=f"inb{cc}", bufs=4, name="inp_b")
            nc.sync.dma_start(out=inp_b, in_=xaug[:, rt, cols])
        else:
            inp_f = sbA.tile([128, bk], BF16, tag=f"inf{cc}", bufs=4, name="inp_f")
            nc.sync.dma_start(out=inp_f, in_=O[l - 1][:, k, cols])
            inp_b = sbA.tile([128, bk], BF16, tag=f"inb{cc}", bufs=4, name="inp_b")
            nc.sync.dma_start(out=inp_b, in_=O[l - 1][:, rt, cols])
        return inp_f, inp_b

    def lstm_step(cc, l, k, inp_f, inp_b):
        c0 = cc * bk
        cols = slice(c0, c0 + bk)
        rt = t_len - 1 - k

        P_ifo = ps.tile([128, 3, bk], F32, tag=f"pifo{cc}")
        P_g = ps.tile([128, bk], F32, tag=f"pg{cc}")

        def gate_mms(out_ap, g, j):
            calls = []  # (out, lhsT, rhs, tile_position, partition_range)
            w = l0w_sb if l == 0 else pw_sb
            wf = w[:, g, 0, :] if l == 0 else w[:, l - 1, g, 0, :]
            wb = w[:, g, 1, :] if l == 0 else w[:, l - 1, g, 1, :]
            calls.append((out_ap[0:64, :], wf, inp_f, (0, 0), (0, 64)))
            calls.append((out_ap[64:128, :], wb, inp_b, (0, 64), (64, 128)))
            if k > 0:
                calls.append((out_ap, rw_sb[:, l, g, :], h_prev[cc], None,
                              (0, 128)))
            n = len(calls)
            for i, (o, lh, rh, tp, rng) in enumerate(calls):
                # start: this call's partitions not all covered by earlier calls
                covered = set()
                for _, _, _, _, r in calls[:i]:
                    covered.update(range(*r))
                start = not set(range(*rng)).issubset(covered)
                # stop: no later call touches this call's partitions
                stop = not any(max(rng[0], r[0]) < min(rng[1], r[1])
                               for _, _, _, _, r in calls[i + 1:])
                # skip_group_check: the executing-sim group checker
                # mis-addresses partition-based PSUM offsets (tensor rows
                # != 16KB); data semantics are still simulated exactly.
                nc.tensor.matmul(o, lh, rh, start=start, stop=stop,
                                 tile_position=tp, skip_group_check=True)

        for j, g in enumerate(SIG_GATES):
            gate_mms(P_ifo[:, j, :], g, j)
        gate_mms(P_g, TANH_GATE, None)

        S_ifo = sbB.tile([128, 3, bk], F32, tag=f"sifo{cc}", bufs=3)
        S_g = sbB.tile([128, bk], F32, tag=f"sg{cc}")

        def sig(j):
            if l > 0:
                idx = (l - 1) * 3 + j
                nc.scalar.activation(out=S_ifo[:, j, :], in_=P_ifo[:, j, :],
                                     func=AF.Sigmoid,
                                     bias=br_sb[:, idx:idx + 1])
            else:
                nc.scalar.activation(out=S_ifo[:, j, :], in_=P_ifo[:, j, :],
                                     func=AF.Sigmoid)

        sig(0)                                                    # i
        if l > 0:
            nc.scalar.activation(out=S_g, in_=P_g, func=AF.Tanh,
                                 bias=gb_sb[:, l - 1:l])
        else:
            nc.scalar.activation(out=S_g, in_=P_g, func=AF.Tanh)
        sig(1)                                                    # f
        sig(2)                                                    # o

        if k == 0:
            c = sbS.tile([128, bk], F32, tag=f"c{cc}")
            c_st[cc] = c
            nc.vector.tensor_mul(c, S_ifo[:, 0, :], S_g)          # c = i*g
        else:
            c = c_st[cc]
            tmp = sbB.tile([128, bk], F32, tag=f"tmp{cc}")
            nc.gpsimd.tensor_mul(tmp, S_ifo[:, 0, :], S_g)        # i*g (POOL)
            nc.vector.tensor_mul(c, S_ifo[:, 1, :], c)            # f*c
            nc.vector.tensor_add(c, c, tmp)
        return S_ifo, c

    def lstm_step_ph2(cc, l, k, S_ifo, c):
        # second phase emitted after the other chunk's phase 1 so the
        # ScalarE FIFO never head-of-line blocks on tanh(c) while the other
        # chunk's (ready) sigmoid sits behind it
        c0 = cc * bk
        cols = slice(c0, c0 + bk)
        rt = t_len - 1 - k
        Tc = sbB.tile([128, bk], F32, tag=f"tc{cc}")
        nc.scalar.activation(out=Tc, in_=c, func=AF.Tanh)
        h = sbA.tile([128, bk], F32, tag=f"h{cc}")
        nc.vector.tensor_mul(h, S_ifo[:, 2, :], Tc)               # h = o*tanh(c)
        h_prev[cc] = h

        # cast + store time-ordered halves: fwd half at t=k, bwd half at t=rt
        h_bf = sbA.tile([128, bk], BF16, tag=f"hbf{cc}")
        nc.gpsimd.tensor_copy(out=h_bf, in_=h)
        nc.sync.dma_start(out=O[l][0:64, k, cols], in_=h_bf[0:64, :])
        nc.sync.dma_start(out=O[l][64:128, rt, cols], in_=h_bf[64:128, :])

    PF = min(2, t_len - 1)
    for l in range(NL):
        pend = {}
        for kk in range(PF):
            for cc in range(CHUNKS):
                pend[(cc, kk)] = issue_inp(cc, l, kk)
        for k in range(t_len):
            ph1 = {}
            for cc in range(CHUNKS):
                if k + PF < t_len:
                    pend[(cc, k + PF)] = issue_inp(cc, l, k + PF)
                inp_f, inp_b = pend.pop((cc, k))
                ph1[cc] = lstm_step(cc, l, k, inp_f, inp_b)
            for cc in range(CHUNKS):
                S_ifo, c = ph1[cc]
                lstm_step_ph2(cc, l, k, S_ifo, c)

    # ---- final stage: relu + residual + LayerNorm + transpose to natural ----
    sums = [sbS.tile([fb, nb, t_len], F32, tag=f"sums{cc}", name=f"sums{cc}")
            for cc in range(CHUNKS)]
    sumsq = [sbS.tile([fb, nb, t_len], F32, tag=f"sumsq{cc}", name=f"sumsq{cc}")
             for cc in range(CHUNKS)]

    def issue_fin(cc, t):
        c0 = cc * bk
        cols = slice(c0, c0 + bk)
        o2t = sbA.tile([128, bk], BF16, tag=f"inf{cc}", bufs=4, name="o2t")
        nc.sync.dma_start(out=o2t, in_=O[NL - 1][:, t, cols])
        xt = sbA.tile([9, bk], F32, tag=f"inb{cc}", bufs=4, name="xt")
        nc.sync.dma_start(out=xt, in_=xaug[:, t, cols])
        return o2t, xt

    def final_t(cc, t, zs, o2t, xt):
        c0 = cc * bk
        cols = slice(c0, c0 + bk)
        relu4 = sbB.tile([128, bk], F32, tag=f"relu{cc}")
        nc.gpsimd.tensor_scalar_max(relu4, o2t, 0.0)
        # one accumulation group for the whole bank: transpose overwrites its
        # quarter (pending-zero from the single start), residual accumulates
        psZ = ps.tile([fb, nb, 128], F32, tag=f"pg{cc}")
        for bi in range(nb):
            bs = slice(bi * fb, (bi + 1) * fb)
            nc.tensor.matmul(psZ[:, bi, :], relu4[:, bs], ident_sb,
                             is_transpose=True, start=(bi == 0), stop=False,
                             skip_group_check=True)
            nc.tensor.matmul(psZ[:, bi, :], xt[:, bs], wres_sb,
                             start=False, stop=(bi == nb - 1),
                             skip_group_check=True)
        z = sbZ.tile([fb, nb, 128], F32, tag=f"z{cc}")
        z2 = sbB.tile([fb, nb, 128], F32, tag=f"z2{cc}")
        for bi in range(nb):
            nc.scalar.activation(out=z[:, bi, :], in_=psZ[:, bi, :],
                                 func=AF.Identity,
                                 accum_out=sums[cc][:, bi, t:t + 1])
            nc.scalar.activation(out=z2[:, bi, :], in_=psZ[:, bi, :],
                                 func=AF.Square,
                                 accum_out=sumsq[cc][:, bi, t:t + 1])
        zs.append((t, z))

    def final_strip_norm(cc, t0, zs):
        c0 = cc * bk
        ss = slice(t0, t0 + strip)
        mu = sbB.tile([fb, nb, strip], F32, tag=f"mu{cc}")
        nc.vector.tensor_scalar_mul(mu, sums[cc][:, :, ss], 1.0 / D2)
        var = sbB.tile([fb, nb, strip], F32, tag=f"var{cc}")
        nc.vector.tensor_scalar_mul(var, sumsq[cc][:, :, ss], 1.0 / D2)
        mu2 = sbB.tile([fb, nb, strip], F32, tag=f"mu2{cc}")
        nc.vector.tensor_mul(mu2, mu, mu)
        nc.vector.tensor_sub(var, var, mu2)
        sd = sbB.tile([fb, nb, strip], F32, tag=f"sd{cc}")
        nc.scalar.activation(out=sd, in_=var, func=AF.Sqrt,
                             bias=eps_sb[0:fb, 0:1])
        rstd = sbB.tile([fb, nb, strip], F32, tag=f"rstd{cc}")
        nc.vector.reciprocal(rstd, sd)
        nmr = sbB.tile([fb, nb, strip], F32, tag=f"nmr{cc}")
        nc.vector.scalar_tensor_tensor(nmr, mu, -1.0, rstd,
                                       op0=OP.mult, op1=OP.mult)
        for (t, z) in zs:
            ti = t - t0
            for bi in range(nb):
                on = sbA.tile([fb, 128], F32, tag=f"on{cc}")
                nc.vector.tensor_scalar(on, z[:, bi, :],
                                        rstd[:, bi, ti:ti + 1],
                                        nmr[:, bi, ti:ti + 1],
                                        op0=OP.mult, op1=OP.add)
                if apply_gb:
                    nc.vector.tensor_mul(on, on, gamma_sb)
                    nc.vector.tensor_add(on, on, beta_sb)
                b0 = c0 + bi * fb
                nc.sync.dma_start(out=out_d[b0:b0 + fb, t, :], in_=on)

    fpend = {}
    for tt in range(PF):
        for cc in range(CHUNKS):
            fpend[(cc, tt)] = issue_fin(cc, tt)
    for t0 in range(0, t_len, strip):
        zstrip = [[] for _ in range(CHUNKS)]
        for t in range(t0, t0 + strip):
            for cc in range(CHUNKS):
                if t + PF < t_len:
                    fpend[(cc, t + PF)] = issue_fin(cc, t + PF)
                o2t, xt = fpend.pop((cc, t))
                final_t(cc, t, zstrip[cc], o2t, xt)
        for cc in range(CHUNKS):
            final_strip_norm(cc, t0, zstrip[cc])


def build(apply_gb=False, bc=BC, t_len=T, num_devices=NCORES):
    nc = bacc.Bacc("TRN2", target_bir_lowering=False, debug=False,
                   num_devices=num_devices)
    fb = min(128, bc // CHUNKS)
    D = {}

    def inp(name, shape, dtype=F32):
        D[name] = nc.dram_tensor(name, shape, dtype, kind="ExternalInput").ap()

    inp("xaug", [9, t_len, bc])
    inp("rw", [128, NL, 4, 128])
    inp("pw", [128, NL - 1, 4, 2, 64], BF16)
    inp("l0w", [9, 4, 2, 64])
    inp("br", [128, (NL - 1) * 3])
    inp("gb", [128, NL - 1])
    inp("wres", [9, 128])
    inp("ident", [128, 128])
    if apply_gb:
        inp("gammab", [fb, 128])
        inp("betab", [fb, 128])
    for i in range(NL):
        D[f"o{i}"] = nc.dram_tensor(f"o{i}", [128, t_len, bc], BF16).ap()
    D["out"] = nc.dram_tensor("out", [bc, t_len, 128], F32,
                              kind="ExternalOutput").ap()

    with tile.TileContext(nc) as tc:
        with ExitStack() as ctx:
            _emit(nc, tc, ctx, D, apply_gb, bc, t_len)
    nc.compile()
    return nc


_BUILD_CACHE = {}


def kernel(x, w_ih, w_hh, b_ih, b_hh, w_res, b_res, ln_gamma, ln_beta):
    ln_gamma = np.asarray(ln_gamma, np.float32)
    ln_beta = np.asarray(ln_beta, np.float32)
    apply_gb = not (np.all(ln_gamma == 1.0) and np.all(ln_beta == 0.0))

    shared, xaug_cores = _host_prep(x, w_ih, w_hh, b_ih, b_hh, w_res, b_res,
                                    NCORES, BC)
    if apply_gb not in _BUILD_CACHE:
        _BUILD_CACHE[apply_gb] = build(apply_gb)
    nc = _BUILD_CACHE[apply_gb]

    in_maps = []
    for c in range(NCORES):
        m = dict(shared)
        m["xaug"] = xaug_cores[c]
        if apply_gb:
            fb = min(128, BC // CHUNKS)
            m["gammab"] = np.ascontiguousarray(
                np.broadcast_to(ln_gamma, (fb, 128)).astype(np.float32))
            m["betab"] = np.ascontiguousarray(
                np.broadcast_to(ln_beta, (fb, 128)).astype(np.float32))
        in_maps.append(m)

    res = run_bass_kernel_spmd(nc, in_maps, core_ids=list(range(NCORES)))
    out = np.concatenate([res.results[c]["out"] for c in range(NCORES)], axis=0)
    return np.ascontiguousarray(out.astype(np.float32))



# revision 2
# speedup vs baseline: 1.9526x; 1.9526x over previous
"""Trainium2 Bass kernel for a 3-layer BiLSTM + ReLU + residual + LayerNorm.

v2 strategy (pure data parallel over 8 cores, 1024 batch rows per core):
  * Transposed on-chip layout: features on partitions (fwd 0:64 / bwd 64:128
    fused), batch on the free dim; two interleaved batch chunks of 512.
  * All matmuls bf16 (1 cycle/row on PE): per-gate recurrent matmul is one
    block-diagonal 128x128; layer-0 input projection is one K=18 matmul per
    gate (both directions + bias rows folded in); layers 1-2 use two M=64
    column-half matmuls per gate.  Recurrent matmuls are emitted first and
    the next step's projections are emitted after the activations so the PE
    queue never head-of-line blocks on the h dependency.
  * ScalarE does the 4 gate activations (per-gate bias APs) + tanh(c);
    i*g runs on GpSimd, f*c / accumulate / h=o*tanh(c) on VectorE; h is
    produced directly as bf16 into an 8-step staging ring that doubles as
    the recurrent-matmul rhs and the per-strip layer-output DMA source.
  * All HBM traffic is strip-batched (8 timesteps per DMA, backward halves
    via negative-stride APs): ~300 DMAs total instead of ~2300.
  * Final stage per 8-step strip: residual via one K=9 matmul, z = relu+res
    on VectorE, z^2 on GpSimd, LN sums via ones-column accumulating matmuls
    (scaled 1/128), tiny PE transposes for the stats, per-batch-row
    normalize split between ScalarE (scale/bias APs) and VectorE.
"""

from contextlib import ExitStack

import numpy as np
import ml_dtypes

import concourse.bacc as bacc
import concourse.tile as tile
from concourse import mybir
from concourse.bass_utils import run_bass_kernel_spmd

F32 = mybir.dt.float32
BF16 = mybir.dt.bfloat16
AF = mybir.ActivationFunctionType
OP = mybir.AluOpType

NCORES = 8
BC = 1024               # batch rows per core
CHUNKS = 2
T = 64
H = 64
NL = 3
D2 = 2 * H              # 128
LN_EPS = 1e-5
SG = 8                  # timesteps per DMA strip group

# PyTorch gate order: i, f, g, o
GI, GF, GG, GO = 0, 1, 2, 3


def _host_prep(x, w_ih, w_hh, b_ih, b_hh, w_res, b_res, ncores, bc):
    """Matmul-ready weight layouts (shared across cores) + per-core inputs."""
    x = np.asarray(x, np.float32)
    w_ih = np.asarray(w_ih, np.float32)
    w_hh = np.asarray(w_hh, np.float32)
    bias = np.asarray(b_ih, np.float32) + np.asarray(b_hh, np.float32)  # (NL,2,4H)
    w_res = np.asarray(w_res, np.float32)
    b_res = np.asarray(b_res, np.float32)
    t_len = x.shape[1]

    # Recurrent lhsT, K-major: rw[k, l, g, m] (block-diagonal over directions)
    rw = np.zeros((128, NL, 4, 128), np.float32)
    for l in range(NL):
        for g in range(4):
            gs = slice(g * H, (g + 1) * H)
            rw[0:64, l, g, 0:64] = w_hh[l, 0, gs, :].T
            rw[64:128, l, g, 64:128] = w_hh[l, 1, gs, :].T
    rw = rw.astype(ml_dtypes.bfloat16)

    # Input-projection lhsT for layers 1,2: pw[k, l-1, g, d, m]
    pw = np.zeros((128, NL - 1, 4, 2, 64), np.float32)
    for l in (1, 2):
        for g in range(4):
            gs = slice(g * H, (g + 1) * H)
            for d in range(2):
                pw[:, l - 1, g, d, :] = w_ih[l, d, gs, :].T
    pw = pw.astype(ml_dtypes.bfloat16)

    # Layer-0 projection lhsT, both directions + bias rows, block-diagonal:
    # rows 0:8 fwd weights, row 8 fwd bias, rows 9:17 bwd weights, row 17 bwd
    l0w = np.zeros((18, 4, 128), np.float32)
    for g in range(4):
        gs = slice(g * H, (g + 1) * H)
        l0w[0:8, g, 0:64] = w_ih[0, 0, gs, 0:8].T
        l0w[8, g, 0:64] = bias[0, 0, gs]
        l0w[9:17, g, 64:128] = w_ih[0, 1, gs, 0:8].T
        l0w[17, g, 64:128] = bias[0, 1, gs]
    l0w = l0w.astype(ml_dtypes.bfloat16)

    # per-partition gate biases for layers 1,2 (fused dirs): br[p, l-1, g]
    br = np.zeros((128, NL - 1, 4), np.float32)
    for l in (1, 2):
        for g in range(4):
            gs = slice(g * H, (g + 1) * H)
            br[0:64, l - 1, g] = bias[l, 0, gs]
            br[64:128, l - 1, g] = bias[l, 1, gs]

    # residual rhs: wres[k, f] = w_res[f, k], row 8 = b_res
    wres = np.zeros((9, 128), np.float32)
    wres[0:8, :] = w_res.T
    wres[8, :] = b_res
    wres = wres.astype(ml_dtypes.bfloat16)

    # ones-column lhsT for the LN sum matmuls: onescube[p, j, m] = (m==j)/128
    onescube = np.zeros((128, SG, SG), np.float32)
    for j in range(SG):
        onescube[:, j, j] = 1.0 / D2
    onescube = onescube.astype(ml_dtypes.bfloat16)

    ident = np.eye(128, dtype=np.float32).astype(ml_dtypes.bfloat16)

    # Per-core transposed-augmented input (bf16): xaug[k, t, b]
    xaug_cores = []
    for c in range(ncores):
        xc = x[c * bc:(c + 1) * bc]              # (bc, T, 8)
        xa = np.empty((9, t_len, bc), np.float32)
        xa[0:8] = xc.transpose(2, 1, 0)
        xa[8] = 1.0
        xaug_cores.append(xa.astype(ml_dtypes.bfloat16))

    shared = dict(rw=rw, pw=pw, l0w=l0w, br=br, wres=wres,
                  onescube=onescube, ident=ident)
    return shared, xaug_cores


def _emit(nc, tc, ctx, D, apply_gb, bc, t_len):
    bk = bc // CHUNKS
    nb = bk // 128            # natural-layout 128-row blocks per chunk
    ngrp = t_len // SG

    sbC = ctx.enter_context(tc.tile_pool(name="consts", bufs=1))
    sbA = ctx.enter_context(tc.tile_pool(name="inps", bufs=1))
    sbB = ctx.enter_context(tc.tile_pool(name="work", bufs=1))
    sbS = ctx.enter_context(tc.tile_pool(name="state", bufs=1))
    ps = ctx.enter_context(tc.tile_pool(name="ps", bufs=1, space="PSUM"))

    def const_tile(shape, dtype, key):
        t = sbC.tile(shape, dtype, name=f"c_{key}", tag=f"c_{key}")
        nc.sync.dma_start(out=t, in_=D[key])
        return t

    rw_sb = const_tile([128, NL, 4, 128], BF16, "rw")
    pw_sb = const_tile([128, NL - 1, 4, 2, 64], BF16, "pw")
    l0w_sb = const_tile([18, 4, 128], BF16, "l0w")
    br_sb = const_tile([128, NL - 1, 4], F32, "br")
    wres_sb = const_tile([9, 128], BF16, "wres")
    ones_sb = const_tile([128, SG, SG], BF16, "onescube")
    ident_sb = const_tile([128, 128], BF16, "ident")
    gamma_sb = beta_sb = None
    if apply_gb:
        gamma_sb = const_tile([128, 128], F32, "gammab")
        beta_sb = const_tile([128, 128], F32, "betab")
    eps_sb = sbC.tile([128, 1], F32)
    nc.vector.memset(eps_sb, LN_EPS)

    O = [D[f"o{i}"] for i in range(NL)]
    xaug = D["xaug"]
    out_d = D["out"]

    cols = [slice(cc * bk, (cc + 1) * bk) for cc in range(CHUNKS)]

    # ---------------- LSTM layers ----------------

    def issue_group(l, cc, grp):
        k0 = grp * SG
        lo = t_len - k0 - SG
        hi = t_len - k0
        if l == 0:
            xa = sbA.tile([18, SG, bk], BF16, tag=f"inF{cc}", bufs=2, name="xa")
            nc.sync.dma_start(out=xa[0:9], in_=xaug[:, k0:k0 + SG, cols[cc]])
            nc.sync.dma_start(out=xa[9:18],
                              in_=xaug[:, lo:hi, cols[cc]][:, ::-1, :])
            return (xa, None)
        inF = sbA.tile([128, SG, bk], BF16, tag=f"inF{cc}", bufs=2, name="inF")
        nc.sync.dma_start(out=inF, in_=O[l - 1][:, k0:k0 + SG, cols[cc]])
        inB = sbA.tile([128, SG, bk], BF16, tag=f"inB{cc}", bufs=2, name="inB")
        nc.sync.dma_start(out=inB,
                          in_=O[l - 1][:, lo:hi, cols[cc]][:, ::-1, :])
        return (inF, inB)

    def emit_proj(l, cc, P, tiles, j, k):
        # input projections for step k (independent of the recurrence)
        stop = (k == 0)   # no recurrent matmul at k==0
        if l == 0:
            xa = tiles[0]
            for g in range(4):
                nc.tensor.matmul(P[:, g, :], l0w_sb[:, g, :], xa[:, j, :],
                                 start=True, stop=stop, skip_group_check=True)
        else:
            inF, inB = tiles
            for g in range(4):
                nc.tensor.matmul(P[0:64, g, :], pw_sb[:, l - 1, g, 0, :],
                                 inF[:, j, :], start=True, stop=stop,
                                 tile_position=(0, 0), skip_group_check=True)
                nc.tensor.matmul(P[64:128, g, :], pw_sb[:, l - 1, g, 1, :],
                                 inB[:, j, :], start=True, stop=stop,
                                 tile_position=(0, 64), skip_group_check=True)

    h_prev = [None] * CHUNKS
    c_st = [None] * CHUNKS
    stage_cur = [None] * CHUNKS

    for l in range(NL):
        pend = {}
        for cc in range(CHUNKS):
            pend[(cc, 0)] = issue_group(l, cc, 0)
        P_cur = [None] * CHUNKS
        for cc in range(CHUNKS):
            P_cur[cc] = ps.tile([128, 4, bk], F32, tag=f"p{cc}", name="P")
            emit_proj(l, cc, P_cur[cc], pend[(cc, 0)], 0, 0)

        for k in range(t_len):
            j = k % SG
            grp = k // SG
            if j == 0:
                if grp + 1 < ngrp:
                    for cc in range(CHUNKS):
                        pend[(cc, grp + 1)] = issue_group(l, cc, grp + 1)
                for cc in range(CHUNKS):
                    stage_cur[cc] = sbS.tile([128, SG, bk], BF16,
                                             tag=f"st{cc}", bufs=2,
                                             name="stage")
            S_os = [None] * CHUNKS
            for cc in range(CHUNKS):
                P = P_cur[cc]
                if k > 0:
                    for g in range(4):
                        nc.tensor.matmul(P[:, g, :], rw_sb[:, l, g, :],
                                         h_prev[cc], start=False, stop=True,
                                         skip_group_check=True)

                def bias(g):
                    if l == 0:
                        return 0.0
                    return br_sb[:, l - 1, g:g + 1]

                S_if = sbB.tile([128, 2, bk], F32, tag=f"sif{cc}", bufs=2,
                                name="S_if")
                S_g = sbB.tile([128, bk], F32, tag=f"sg{cc}", bufs=2,
                               name="S_g")
                S_o = sbB.tile([128, bk], BF16, tag=f"so{cc}", bufs=2,
                               name="S_o")
                nc.scalar.activation(out=S_if[:, 0, :], in_=P[:, GI, :],
                                     func=AF.Sigmoid, bias=bias(GI))
                nc.scalar.activation(out=S_g, in_=P[:, GG, :],
                                     func=AF.Tanh, bias=bias(GG))
                nc.scalar.activation(out=S_if[:, 1, :], in_=P[:, GF, :],
                                     func=AF.Sigmoid, bias=bias(GF))
                nc.scalar.activation(out=S_o, in_=P[:, GO, :],
                                     func=AF.Sigmoid, bias=bias(GO))
                if k == 0:
                    c = sbS.tile([128, bk], F32, tag=f"c{cc}", name="c")
                    c_st[cc] = c
                    nc.vector.tensor_mul(c, S_if[:, 0, :], S_g)
                else:
                    c = c_st[cc]
                    tmp = sbB.tile([128, bk], F32, tag=f"tmp{cc}", bufs=2,
                                   name="tmp")
                    nc.gpsimd.tensor_mul(tmp, S_if[:, 0, :], S_g)
                    nc.vector.tensor_mul(c, S_if[:, 1, :], c)
                    nc.vector.tensor_add(c, c, tmp)
                S_os[cc] = S_o
            for cc in range(CHUNKS):
                Tc = sbB.tile([128, bk], BF16, tag=f"tc{cc}", bufs=2,
                              name="Tc")
                nc.scalar.activation(out=Tc, in_=c_st[cc], func=AF.Tanh)
                hslot = stage_cur[cc][:, j, :]
                nc.vector.tensor_mul(hslot, S_os[cc], Tc)
                h_prev[cc] = hslot
            # next step's projections (prefetched past the h dependency)
            if k + 1 < t_len:
                jn = (k + 1) % SG
                gn = (k + 1) // SG
                for cc in range(CHUNKS):
                    P_cur[cc] = ps.tile([128, 4, bk], F32, tag=f"p{cc}",
                                        name="P")
                    emit_proj(l, cc, P_cur[cc], pend[(cc, gn)], jn, k + 1)
            if j == SG - 1:
                k0 = grp * SG
                lo = t_len - k0 - SG
                hi = t_len - k0
                for cc in range(CHUNKS):
                    nc.sync.dma_start(out=O[l][0:64, k0:k0 + SG, cols[cc]],
                                      in_=stage_cur[cc][0:64, :, :])
                    nc.sync.dma_start(
                        out=O[l][64:128, lo:hi, cols[cc]][:, ::-1, :],
                        in_=stage_cur[cc][64:128, :, :])
                if grp >= 1:
                    pend.pop((0, grp - 1), None)
                    pend.pop((1, grp - 1), None)

    # ---------------- final stage: relu + residual + LayerNorm ----------------
    # PSUM scratch per chunk reuses the gate tile (4 banks):
    #   slots 0/1: residual ping-pong, then bf16 z-transpose / stats regions
    #   slot 2: LN mean accumulator [0:8]   slot 3: LN sq-mean accumulator

    fpend = {}
    for cc in range(CHUNKS):
        fpend[(cc, 0)] = None

    def issue_fin(cc, grp):
        t0 = grp * SG
        o2 = sbA.tile([128, SG, bk], BF16, tag=f"inF{cc}", bufs=2, name="o2")
        nc.sync.dma_start(out=o2, in_=O[NL - 1][:, t0:t0 + SG, cols[cc]])
        xa9 = sbA.tile([9, SG, bk], BF16, tag=f"inB{cc}", bufs=2, name="xa9")
        nc.sync.dma_start(out=xa9, in_=xaug[:, t0:t0 + SG, cols[cc]])
        return o2, xa9

    for cc in range(CHUNKS):
        fpend[(cc, 0)] = issue_fin(cc, 0)

    for grp in range(ngrp):
        t0 = grp * SG
        if grp + 1 < ngrp:
            for cc in range(CHUNKS):
                fpend[(cc, grp + 1)] = issue_fin(cc, grp + 1)
        for cc in range(CHUNKS):
            o2, xa9 = fpend.pop((cc, grp))
            relu = sbS.tile([128, SG, bk], BF16, tag=f"st{cc}", bufs=2,
                            name="relu")
            nc.gpsimd.tensor_scalar_max(relu, o2, 0.0)
            scr = ps.tile([128, 4, bk], F32, tag=f"p{cc}", name="scr")
            zs = sbB.tile([128, SG, bk], BF16, tag=f"zs{cc}", name="zs")
            for jt in range(SG):
                res = scr[:, jt % 2, :]
                nc.tensor.matmul(res, wres_sb, xa9[:, jt, :],
                                 start=True, stop=True, skip_group_check=True)
                nc.vector.tensor_add(zs[:, jt, :], relu[:, jt, :], res)
                zq = sbB.tile([128, bk], BF16, tag=f"zq{cc}", bufs=2,
                              name="zq")
                nc.gpsimd.tensor_mul(zq, zs[:, jt, :], zs[:, jt, :])
                nc.tensor.matmul(scr[0:8, 2, :], ones_sb[:, jt, :],
                                 zs[:, jt, :], start=(jt == 0),
                                 stop=(jt == SG - 1), skip_group_check=True)
                nc.tensor.matmul(scr[0:8, 3, :], ones_sb[:, jt, :], zq,
                                 start=(jt == 0), stop=(jt == SG - 1),
                                 skip_group_check=True)
            # stats: mu/sqm [8, bk] -> natural layout [128, nb, 8]
            musq = sbB.tile([8, 2, bk], BF16, tag=f"ms{cc}", name="musq")
            nc.scalar.activation(out=musq, in_=scr[0:8, 2:4, :],
                                 func=AF.Identity)
            sv = scr[:, 0, :].bitcast(BF16)      # [128, 2*bk] bf16 view
            for bi in range(nb):
                b0 = bi * 128
                nc.tensor.matmul(sv[:, bi * 16:bi * 16 + 8],
                                 musq[:, 0, b0:b0 + 128], ident_sb[0:8, 0:8],
                                 is_transpose=True, start=True, stop=True,
                                 skip_group_check=True)
                nc.tensor.matmul(sv[:, bi * 16 + 8:bi * 16 + 16],
                                 musq[:, 1, b0:b0 + 128], ident_sb[0:8, 0:8],
                                 is_transpose=True, start=True, stop=True,
                                 skip_group_check=True)
            snat = sbB.tile([128, nb, 16], BF16, tag=f"sn{cc}", name="snat")
            nc.scalar.activation(out=snat,
                                 in_=sv[:, 0:nb * 16].rearrange(
                                     "p (a c) -> p a c", a=nb),
                                 func=AF.Identity)
            mu_nat = snat[:, :, 0:8]
            sq_nat = snat[:, :, 8:16]
            mu2 = sbB.tile([128, nb, 8], F32, tag=f"mu2{cc}", name="mu2")
            nc.vector.tensor_mul(mu2, mu_nat, mu_nat)
            var = sbB.tile([128, nb, 8], F32, tag=f"var{cc}", name="var")
            nc.vector.tensor_sub(var, sq_nat, mu2)
            sd = sbB.tile([128, nb, 8], F32, tag=f"sd{cc}", name="sd")
            nc.scalar.activation(out=sd, in_=var, func=AF.Sqrt,
                                 bias=eps_sb)
            rstd = sbB.tile([128, nb, 8], F32, tag=f"rstd{cc}", name="rstd")
            nc.vector.reciprocal(rstd, sd)
            nmr = sbB.tile([128, nb, 8], F32, tag=f"nmr{cc}", name="nmr")
            nc.vector.scalar_tensor_tensor(nmr, mu_nat, -1.0, rstd,
                                           op0=OP.mult, op1=OP.mult)
            outst = [sbB.tile([128, SG, 128], F32, tag=f"os{cc}{bi}",
                              name="outst")
                     for bi in range(nb)]
            for jt in range(SG):
                zv = scr[:, jt % 2, :].bitcast(BF16)   # [128, 2*bk] bf16
                for bi in range(nb):
                    b0 = bi * 128
                    nc.tensor.matmul(zv[:, b0:b0 + 128], zs[:, jt, b0:b0 + 128],
                                     ident_sb, is_transpose=True,
                                     start=True, stop=True,
                                     skip_group_check=True)
                for bi in range(nb):
                    b0 = bi * 128
                    dst = outst[bi][:, jt, :]
                    if bi < 3:
                        nc.scalar.activation(out=dst, in_=zv[:, b0:b0 + 128],
                                             func=AF.Identity,
                                             scale=rstd[:, bi, jt:jt + 1],
                                             bias=nmr[:, bi, jt:jt + 1])
                    else:
                        nc.vector.tensor_scalar(dst, zv[:, b0:b0 + 128],
                                                rstd[:, bi, jt:jt + 1],
                                                nmr[:, bi, jt:jt + 1],
                                                op0=OP.mult, op1=OP.add)
                    if apply_gb:
                        nc.vector.tensor_mul(dst, dst, gamma_sb)
                        nc.vector.tensor_add(dst, dst, beta_sb)
            for bi in range(nb):
                b0 = cc * bk + bi * 128
                nc.sync.dma_start(out=out_d[b0:b0 + 128, t0:t0 + SG, :],
                                  in_=outst[bi])


def build(apply_gb=False, bc=BC, t_len=T, num_devices=NCORES):
    nc = bacc.Bacc("TRN2", target_bir_lowering=False, debug=False,
                   num_devices=num_devices)
    D = {}

    def inp(name, shape, dtype=F32):
        D[name] = nc.dram_tensor(name, shape, dtype, kind="ExternalInput").ap()

    inp("xaug", [9, t_len, bc], BF16)
    inp("rw", [128, NL, 4, 128], BF16)
    inp("pw", [128, NL - 1, 4, 2, 64], BF16)
    inp("l0w", [18, 4, 128], BF16)
    inp("br", [128, NL - 1, 4])
    inp("wres", [9, 128], BF16)
    inp("onescube", [128, SG, SG], BF16)
    inp("ident", [128, 128], BF16)
    if apply_gb:
        inp("gammab", [128, 128])
        inp("betab", [128, 128])
    for i in range(NL):
        D[f"o{i}"] = nc.dram_tensor(f"o{i}", [128, t_len, bc], BF16).ap()
    D["out"] = nc.dram_tensor("out", [bc, t_len, 128], F32,
                              kind="ExternalOutput").ap()

    with tile.TileContext(nc) as tc:
        with ExitStack() as ctx:
            _emit(nc, tc, ctx, D, apply_gb, bc, t_len)
    nc.compile()
    return nc


_BUILD_CACHE = {}


def kernel(x, w_ih, w_hh, b_ih, b_hh, w_res, b_res, ln_gamma, ln_beta):
    ln_gamma = np.asarray(ln_gamma, np.float32)
    ln_beta = np.asarray(ln_beta, np.float32)
    apply_gb = not (np.all(ln_gamma == 1.0) and np.all(ln_beta == 0.0))

    shared, xaug_cores = _host_prep(x, w_ih, w_hh, b_ih, b_hh, w_res, b_res,
                                    NCORES, BC)
    if apply_gb not in _BUILD_CACHE:
        _BUILD_CACHE[apply_gb] = build(apply_gb)
    nc = _BUILD_CACHE[apply_gb]

    in_maps = []
    for c in range(NCORES):
        m = dict(shared)
        m["xaug"] = xaug_cores[c]
        if apply_gb:
            m["gammab"] = np.ascontiguousarray(
                np.broadcast_to(ln_gamma, (128, 128)).astype(np.float32))
            m["betab"] = np.ascontiguousarray(
                np.broadcast_to(ln_beta, (128, 128)).astype(np.float32))
        in_maps.append(m)

    res = run_bass_kernel_spmd(nc, in_maps, core_ids=list(range(NCORES)))
    out = np.concatenate([res.results[c]["out"] for c in range(NCORES)], axis=0)
    return np.ascontiguousarray(out.astype(np.float32))
